# revision 4
# baseline (speedup 1.0000x reference)
"""Trainium2 Bass kernel for a dense transformer encoder block (fp8 v2).

Problem: x[4, 2048, 768], LayerNorm over the *sequence* axis (per-feature
stats), 12-head self-attention, exact-GELU MLP (3072), two residuals.

Sharding: 8 cores = 4 batches x 2 sequence-halves (as v1). Each core gets
its batch's full sequence (own half first), computes LN1 and full K/V
locally, Q/attention/MLP for its own 1024 rows. Only collective: 6 KB
pairwise AllReduce of LN2 partial sums.

v2 changes vs the bf16 baseline:
- All 128-contraction matmuls run fp8e4 DoubleRow (two k-tiles per
  instruction): V/Q/K projections, AV, Wo, both MLP matmuls. Weights are
  host-quantized fp8 at x4096 scale; activations are quantized on device
  (DVE) at x1. Scores stay bf16 (64-deep contraction can't pair) and rely
  on 2x PE row-group concurrency (head pair at partition bases 0/64).
- Bias folds: bv and bo never touch the device math (softmax rows sum to
  1 so A@(xnWv+bv) = A@(xnWv) + bv; LN2 is invariant to per-feature
  constants) -- b2'' = b2 + bo + bv@wo is added once at the end. The V
  ones-row bias matmuls and their PE cost are gone.
- Act engine runs ONLY softmax exp (fp8 out, feeding AV directly) and
  GELU. Every affine/copy moved to DVE tensor_scalar/tensor_tensor.
- Attention normalization pipelined per head-pair: denominators stream
  out of PSUM by DMA, reciprocal + selector broadcast + normalize run
  under the next pair's matmuls. y is scaled x32 into fp8 (Wo PSUM is
  x131072, removed in the x2 copy).
- LN2 partial stats via DVE bn_stats on x2 (no Act Square pass).
"""

import sys
from collections import deque

for _p in ("/opt/trn_rl_repo",):
    if _p not in sys.path:
        sys.path.append(_p)

import numpy as np

B, N, D, H, KH, MLPD = 4, 2048, 768, 12, 64, 3072
P = 128
DT = D // P  # 6 feature tiles
NO = N // 2  # 1024 rows owned per core
MT = N // P  # 16 m-tiles
MTP = MT // 2  # 8 m-tile pairs
HT = MLPD // P  # 24 hidden tiles
HTP = HT // 2  # 12 hidden tile pairs
CH = 512
OCH = NO // CH  # 2 own-row chunks
NCH = N // CH  # 4 full-row chunks
MPC = CH // P  # 4 m-tiles per chunk
EPS = 1e-6
NC = 8
SW = 4096.0  # fp8 weight scale
SY = 32.0  # fp8 y scale
VP = 80  # padded AV stationary width (65 used; stride must be %16)

_CACHE = {}


def _install_drain_patch(tile_mod):
    """This container's walrus accepts at most ONE semaphore wait on a Drain
    (CTRL_NO_STRUCT) instruction, but TileContext's kernel-tail drain carries
    every outstanding wait. Split them across a chain of Drains."""
    from concourse.vector_clock import ScopedClock

    if getattr(tile_mod.TileContext, "_drain_patched", False):
        return

    def _patched(self, tick_clock, wait_clock):
        nc = self.nc
        drain_inst = nc.sync.drain()
        wait_clock.add_sem_waits(
            drain_inst.ins, ScopedClock({None: tick_clock.global_clock})
        )
        i = drain_inst.ins
        si = i.sync_info
        waits = list(si.on_wait) if si is not None else []
        if len(waits) > 1:
            si.on_wait = waits[:1]
            i.sync_info = si
            cls = type(si)
            for k in range(1, len(waits)):
                d2 = nc.sync.drain()
                d2.ins.sync_info = cls(on_wait=waits[k : k + 1], on_update=[])
        nc.all_engine_barrier()
        popped = nc._tile_sem_poison_stack.pop()
        assert popped is self._sem_poison
        nc.clear_and_free_semaphores(list(self.sems.allocated().values()))
        nc.all_engine_barrier()

    tile_mod.TileContext._drain_and_barrier = _patched
    tile_mod.TileContext._drain_patched = True


def _split_waits(nc, mybir, limit=1):
    """This walrus build encodes at most ONE semaphore wait per instruction
    across several instruction templates. Move excess waits onto preceding
    same-engine NoOps (engine blocks on each in turn - semantically equal)."""
    nops = 0
    for f in nc.m.functions:
        for b in f.blocks:
            insts = b.instructions
            out = []
            changed = False
            for i in insts:
                si = getattr(i, "sync_info", None)
                waits = list(si.on_wait) if si is not None else []
                if len(waits) > limit:
                    changed = True
                    cls = type(si)
                    for k in range(len(waits) - limit):
                        nop = mybir.InstNoOp(
                            name=f"{i.name}_wsplit{k}", ins=[], outs=[]
                        )
                        nop.engine = i.engine
                        nop.sync_info = cls(on_wait=[waits[k]], on_update=[])
                        out.append(nop)
                        nops += 1
                    si.on_wait = waits[len(waits) - limit :]
                    i.sync_info = si
                out.append(i)
            if changed:
                b.instructions = out
    return nops


def _build_bass(sim=False, phases=4, reps=1):
    import concourse.bass as bass
    import concourse.mybir as mybir
    import concourse.tile as tile

    _install_drain_patch(tile)

    f32 = mybir.dt.float32
    f32r = mybir.dt.float32r
    bf16 = mybir.dt.bfloat16
    f8 = mybir.dt.float8e4
    AF = mybir.ActivationFunctionType
    ALU = mybir.AluOpType
    DR = mybir.MatmulPerfMode.DoubleRow

    nc = bass.Bass(num_devices=NC)

    # ---- DRAM I/O ----
    xT_d = nc.dram_tensor("xT", [P, DT, N], bf16, kind="ExternalInput")
    xo131_d = nc.dram_tensor("xo131", [P, DT, NO], bf16, kind="ExternalInput")
    wq_d = nc.dram_tensor("wq8", [P, DT, D], f8, kind="ExternalInput")
    wk_d = nc.dram_tensor("wk8", [P, DT, D], f8, kind="ExternalInput")
    wv_d = nc.dram_tensor("wv8", [P, DT, D], f8, kind="ExternalInput")
    wo_d = nc.dram_tensor("wo8", [P, DT, D], f8, kind="ExternalInput")
    w1_d = nc.dram_tensor("w18", [P, DT, MLPD], f8, kind="ExternalInput")
    w2_d = nc.dram_tensor("w28", [P, HT, D], f8, kind="ExternalInput")
    vecs_d = nc.dram_tensor("vecs", [P, 8, DT], f32, kind="ExternalInput")
    # vecs slots: 0 ln1_w, 1 ln1_b, 2 ln2_w, 3 ln2_b, 4 bq/sqrt(D), 5 bk,
    # 6 unused, 7 b2'' = b2 + bo + bv@wo
    b1_d = nc.dram_tensor("b1", [P, HT], f32, kind="ExternalInput")
    sel_d = nc.dram_tensor("sel", [12, 12, 64], f32r, kind="ExternalInput")
    id_d = nc.dram_tensor("idm", [P, P], bf16, kind="ExternalInput")
    out_d = nc.dram_tensor("outT", [P, DT, NO], f32, kind="ExternalOutput")

    SCL = float(1.0 / np.sqrt(np.float64(D)))
    UNB = float(N) / float(N - 1)

    def body(tc):
        consts = tc.alloc_tile_pool(name="consts", bufs=1, side="left")
        dram = tc.alloc_tile_pool(name="dram", bufs=1, space="DRAM")
        stats = tc.alloc_tile_pool(name="stats", bufs=1, side="left")

        # ---- constants (Act-ring DMAs) ----
        vecs = consts.tile([P, 8, DT], f32)
        nc.scalar.dma_start(out=vecs[:], in_=vecs_d[:])
        ln1w, ln1b = vecs[:, 0, :], vecs[:, 1, :]
        ln2w, ln2b = vecs[:, 2, :], vecs[:, 3, :]
        bqs, bk_, b2pp = vecs[:, 4, :], vecs[:, 5, :], vecs[:, 7, :]
        b1_ = consts.tile([P, HT], f32)
        nc.scalar.dma_start(out=b1_[:], in_=b1_d[:])
        sel_sb = consts.tile([12, 12, 64], f32r)
        nc.scalar.dma_start(out=sel_sb[:], in_=sel_d[:])
        id_sb = consts.tile([P, P], bf16)
        nc.scalar.dma_start(out=id_sb[:], in_=id_d[:])

        cc_in = dram.tile([P, DT, 2], f32)
        cc_out = dram.tile([P, DT, 2], f32)

        # yTn8 lives left-bottom (written during attention, read by Wo)
        p_y = tc.alloc_tile_pool(name="p_y", bufs=1, side="left")
        yTn8 = p_y.tile([P, DT, NO], f8, tag="yTn8")

        p_w2h = tc.alloc_tile_pool(name="p_w2h", bufs=1, side="left")
        w2_sb = p_w2h.tile([P, HT, D], f8, tag="w2")
        p_w8 = tc.alloc_tile_pool(name="p_w8", bufs=3, side="left")

        # ---- V storage (fp8) with ones column at 64, zero pad 65:80 ----
        p_vsb = tc.alloc_tile_pool(name="p_vsb", bufs=1, side="left")
        v_sb = p_vsb.tile([P, MT, H, VP], f8, tag="v_sb")
        nc.vector.memset(v_sb[:, :, :, 64:VP], 0.0)
        nc.vector.memset(v_sb[:, :, :, 64:65], 1.0)

        # ---- q/k outputs (bf16; die after attention) ----
        p_qk = tc.alloc_tile_pool(name="p_qk", bufs=1, side="right")
        qT = p_qk.tile([P, DT, NO], bf16, tag="qT")
        kT = p_qk.tile([P, DT, N], bf16, tag="kT")

        # ================= Phase L: x load + LN1 stats =================
        p_w = tc.alloc_tile_pool(name="p_w", bufs=2, side="right")
        p_xn = tc.alloc_tile_pool(name="p_xn", bufs=1, side="left")
        xnT = p_xn.tile([P, DT, N], f8, tag="xnT")

        p_x = tc.alloc_tile_pool(name="p_x", bufs=1, side="left")
        xT = p_x.tile([P, DT, N], bf16, tag="xT")
        for dt in range(DT):
            for h in range(2):
                eng = nc.sync if (2 * dt + h) % 2 == 0 else nc.scalar
                hs = slice(h * (N // 2), (h + 1) * (N // 2))
                eng.dma_start(out=xT[:, dt, hs], in_=xT_d[:, dt, hs])

        wv_sb = p_w.tile([P, DT, D], f8, tag="wfull", name="wv_sb")
        nc.sync.dma_start(out=wv_sb[:], in_=wv_d[:])
        wq_sb = p_w.tile([P, DT, D], f8, tag="wfull", name="wq_sb")
        nc.sync.dma_start(out=wq_sb[:], in_=wq_d[:])

        mvs = stats.tile([P, DT, 2], f32)
        nsub = N // 512
        bnst = stats.tile([P, nsub, nc.vector.BN_STATS_DIM], f32, tag="bnst")
        asum = stats.tile([P, 2], f32, tag="asum")
        asq = stats.tile([P, 2], f32, tag="asq")
        ascr = stats.tile([P, N], f8, tag="ascr")
        ACT_TILES = (3, 5)
        for i, dt in enumerate(ACT_TILES):
            nc.scalar.activation(
                out=ascr[:], in_=xT[:, dt, :], func=AF.Identity,
                accum_out=asum[:, i : i + 1],
            )
            nc.scalar.activation(
                out=ascr[:], in_=xT[:, dt, :], func=AF.Square,
                accum_out=asq[:, i : i + 1],
            )
        for dt in range(DT):
            if dt in ACT_TILES:
                continue
            xv = xT[:, dt, :].rearrange("p (s n) -> p s n", s=nsub)
            for s in range(nsub):
                nc.vector.bn_stats(out=bnst[:, s, :], in_=xv[:, s, :])
            nc.vector.bn_aggr(out=mvs[:, dt, :], in_=bnst[:])
        for i, dt in enumerate(ACT_TILES):
            nc.vector.tensor_scalar_mul(
                out=mvs[:, dt, 0:1], in0=asum[:, i : i + 1], scalar1=1.0 / N
            )
            nc.vector.tensor_mul(
                out=mvs[:, dt, 1:2], in0=mvs[:, dt, 0:1], in1=mvs[:, dt, 0:1]
            )
            nc.vector.tensor_scalar(
                out=mvs[:, dt, 1:2],
                in0=asq[:, i : i + 1],
                scalar1=1.0 / N,
                scalar2=mvs[:, dt, 1:2],
                op0=ALU.mult,
                op1=ALU.subtract,
            )

        sig = stats.tile([P, DT], f32, tag="sig")
        inv = stats.tile([P, DT], f32, tag="inv")
        sca = stats.tile([P, DT], f32, tag="sca")
        bia = stats.tile([P, DT], f32, tag="bia")
        # 1/sigma = 1/sqrt(var_pop * N/(N-1)); +eps on sigma is far below
        # the quantization noise floor
        nc.scalar.activation(out=sig[:], in_=mvs[:, :, 1], func=AF.Sqrt, scale=UNB)
        nc.vector.reciprocal(out=inv[:], in_=sig[:])
        nc.vector.tensor_mul(out=sca[:], in0=ln1w, in1=inv[:])
        nc.vector.tensor_mul(out=bia[:], in0=mvs[:, :, 0], in1=sca[:])
        nc.vector.tensor_tensor(out=bia[:], in0=ln1b, in1=bia[:], op=ALU.subtract)

        # ============ Phase P1: chunked xn (fp8) + V projection ============
        psV = tc.alloc_tile_pool(name="psV", bufs=8, space="PSUM")
        for ch in range(NCH):
            csl = slice(ch * CH, (ch + 1) * CH)
            for dt in range(DT):
                if ch == 0:
                    nc.scalar.activation(
                        out=xnT[:, dt, csl],
                        in_=xT[:, dt, csl],
                        func=AF.Identity,
                        bias=bia[:, dt : dt + 1],
                        scale=sca[:, dt : dt + 1],
                    )
                else:
                    nc.gpsimd.tensor_scalar(
                        out=xnT[:, dt, csl],
                        in0=xT[:, dt, csl],
                        scalar1=sca[:, dt : dt + 1],
                        scalar2=bia[:, dt : dt + 1],
                        op0=ALU.mult,
                        op1=ALU.add,
                    )
            for mt in range(ch * MPC, (ch + 1) * MPC):
                for c0, cw, h0, hn in ((0, 512, 0, 8), (512, 256, 8, 4)):
                    ps = psV.tile([P, CH], f32, tag="ps", name="psv")
                    for dk in range(0, DT, 2):
                        nc.tensor.matmul(
                            ps[:, :cw],
                            lhsT=xnT[:, dk : dk + 2, mt * P : (mt + 1) * P],
                            rhs=wv_sb[:, dk : dk + 2, c0 : c0 + cw],
                            start=(dk == 0),
                            stop=(dk == DT - 2),
                            perf_mode=DR,
                        )
                    nc.scalar.mul(
                        out=v_sb[:, mt, h0 : h0 + hn, 0:64],
                        in_=ps[:, 0:cw].rearrange("p (h k) -> p h k", h=hn),
                        mul=1.0 / SW,
                    )
        p_x.release()

        # ============ Phase P2: Q^T (own rows; scale 1/sqrt(D)) ============
        for dt in range(DT):
            for ch in range(OCH):
                ps = psV.tile([P, CH], f32, tag="ps", name="psq")
                for dk in range(0, DT, 2):
                    nc.tensor.matmul(
                        ps[:],
                        lhsT=wq_sb[:, dk : dk + 2, dt * P : (dt + 1) * P],
                        rhs=xnT[:, dk : dk + 2, ch * CH : (ch + 1) * CH],
                        start=(dk == 0),
                        stop=(dk == DT - 2),
                        perf_mode=DR,
                    )
                nc.scalar.activation(
                    out=qT[:, dt, ch * CH : (ch + 1) * CH],
                    in_=ps[:],
                    func=AF.Identity,
                    bias=bqs[:, dt : dt + 1],
                    scale=SCL / SW,
                )

        # ============ Phase P3: K^T (all rows; bias bk) ============
        wk_sb = p_w.tile([P, DT, D], f8, tag="wfull", name="wk_sb")
        nc.sync.dma_start(out=wk_sb[:], in_=wk_d[:])
        for dt in range(DT):
            for ch in range(NCH):
                ps = psV.tile([P, CH], f32, tag="ps", name="psk")
                for dk in range(0, DT, 2):
                    nc.tensor.matmul(
                        ps[:],
                        lhsT=wk_sb[:, dk : dk + 2, dt * P : (dt + 1) * P],
                        rhs=xnT[:, dk : dk + 2, ch * CH : (ch + 1) * CH],
                        start=(dk == 0),
                        stop=(dk == DT - 2),
                        perf_mode=DR,
                    )
                nc.scalar.activation(
                    out=kT[:, dt, ch * CH : (ch + 1) * CH],
                    in_=ps[:],
                    func=AF.Identity,
                    bias=bk_[:, dt : dt + 1],
                    scale=1.0 / SW,
                )
        # prefetch wo + scaled residual through freed weight slots
        wo_sb = p_w.tile([P, DT, D], f8, tag="wfull", name="wo_sb")
        nc.sync.dma_start(out=wo_sb[:], in_=wo_d[:])
        xo131 = p_w.tile([P, DT, NO], bf16, tag="wfull", name="xo131_sb")
        nc.sync.dma_start(out=xo131[:], in_=xo131_d[:])
        psV.release()
        p_xn.release()

        if phases == 1:
            nc.sync.dma_start(out=out_d[:], in_=kT[:].bitcast(f32))
            for p in (p_w, p_qk, p_vsb, p_w8, p_w2h, p_y, stats, consts, dram):
                p.release()
            return

        # ================= Phase P4/P5: attention =================
        # Iteration stream (mtp, ch, hh): PE runs 2 score matmuls per step;
        # exp alternates Act (exact, fp8 out) / DVE (fp8 bit-trick: bits =
        # round(s*8*log2e + 55.54), softmax normalization absorbs the ~3%
        # approx error); the AV DoubleRow trails two steps behind so the PE
        # never waits on an in-flight exp. After each head-pair, yp PSUM is
        # evacuated to SBUF on Act immediately (freeing banks), and the
        # normalize chain (den DMA, reciprocal, selector broadcast, muls)
        # is emitted under the NEXT pair's stream.
        p_att = tc.alloc_tile_pool(name="p_att", bufs=1, side="right")
        p_ex = tc.alloc_tile_pool(name="p_ex", bufs=6, side="right")
        p_sty = tc.alloc_tile_pool(name="p_sty", bufs=2, side="right")
        p_yu = tc.alloc_tile_pool(name="p_yu", bufs=4, side="right")
        psA = tc.alloc_tile_pool(name="psA", bufs=1, space="PSUM")

        den = p_att.tile([12, OCH, CH], f32, tag="den", bufs=1, name="den")
        rcd = p_att.tile([12, OCH, CH], f32r, tag="rcd", bufs=1, name="rcd")
        # unwritten head rows must stay finite: the selector matmul contracts
        # all 12 partitions (zero weights), and 0 * inf would poison it
        nc.vector.memset(den[:], 1.0)

        I8 = mybir.dt.int8
        EC1, EC0 = 11.54156, 55.54

        def emit_av(yp, ph, ch, mtp, hh, ex):
            nc.tensor.matmul(
                yp[hh][0:VP, :],
                lhsT=v_sb[:, 2 * mtp : 2 * mtp + 2, 2 * ph + hh, :],
                rhs=ex[:],
                start=(mtp == 0),
                stop=(mtp == MTP - 1),
                perf_mode=DR,
            )

        def emit_norm_a(ph, ch, yus):
            for hh in range(2):
                h = 2 * ph + hh
                nc.sync.dma_start(
                    out=den[h : h + 1, ch, :], in_=yus[hh][64:65, :]
                )
            nc.vector.reciprocal(out=rcd[:], in_=den[:])

        def emit_norm(ph, ch, yus):
            rbt = psA.tile([P, 2, CH], f32, tag="sp", bufs=3, name="rbt")
            for j in range(2):
                nc.tensor.matmul(
                    rbt[0:64, j, :],
                    lhsT=sel_sb[:, 2 * ph + j, :],
                    rhs=rcd[:, ch, :],
                    start=True,
                    stop=True,
                )
            rbs = p_sty.tile([64, 2, CH], f32, tag="rbs", name="rbs")
            nc.scalar.copy(out=rbs[:], in_=rbt[0:64, :, :])
            csl = slice(ch * CH, (ch + 1) * CH)
            nc.gpsimd.tensor_tensor(
                out=yTn8[0:64, ph, csl],
                in0=yus[0][0:64, :],
                in1=rbs[:, 0, :],
                op=ALU.mult,
            )
            sty = p_sty.tile([64, CH], f8, tag="sty", name="sty")
            nc.gpsimd.tensor_tensor(
                out=sty[:],
                in0=yus[1][0:64, :],
                in1=rbs[:, 1, :],
                op=ALU.mult,
            )
            nc.sync.dma_start(out=yTn8[64:128, ph, csl], in_=sty[:])

        pending_norm = None
        for ph in range(DT):
            for ch in range(OCH):
                yp = [
                    psA.tile([P, CH], f32, tag=f"yp{hh}", bufs=1, name=f"yp{hh}")
                    for hh in range(2)
                ]
                its = [(mtp, hh) for mtp in range(MTP) for hh in range(2)]
                pend_av = deque()
                for idx, (mtp, hh) in enumerate(its):
                    base = hh * 64
                    sp = psA.tile([P, 2, CH], f32, tag="sp", bufs=3, name="sp")
                    for j in range(2):
                        mt = 2 * mtp + j
                        nc.tensor.matmul(
                            sp[:, j, :],
                            lhsT=kT[base : base + KH, ph, mt * P : (mt + 1) * P],
                            rhs=qT[base : base + KH, ph, ch * CH : (ch + 1) * CH],
                            start=True,
                            stop=True,
                        )
                    ex = p_ex.tile([P, 2, CH], f8, tag="ex", name="ex")
                    if hh == 0:
                        nc.scalar.activation(out=ex[:], in_=sp[:], func=AF.Exp)
                    else:
                        nc.vector.tensor_scalar(
                            out=ex[:].bitcast(I8),
                            in0=sp[:],
                            scalar1=EC1,
                            scalar2=EC0,
                            op0=ALU.mult,
                            op1=ALU.add,
                        )
                    pend_av.append((mtp, hh, ex))
                    if len(pend_av) > 3:
                        emit_av(yp, ph, ch, *pend_av.popleft())
                    if idx == 1 and pending_norm is not None:
                        emit_norm_a(*pending_norm)
                    if idx == 6 and pending_norm is not None:
                        emit_norm(*pending_norm)
                        pending_norm = None
                while pend_av:
                    emit_av(yp, ph, ch, *pend_av.popleft())
                # evacuate yp -> SBUF right away so the pass pipeline rolls on
                yus = [None, None]
                for hh in range(2):
                    yu = p_yu.tile([P, CH], f32, tag="yu", name="yu")
                    eng = nc.scalar if hh == 0 else nc.vector
                    if hh == 0:
                        nc.scalar.copy(out=yu[0:65, :], in_=yp[hh][0:65, :])
                    else:
                        nc.vector.tensor_copy(out=yu[0:65, :], in_=yp[hh][0:65, :])
                    yus[hh] = yu
                pending_norm = (ph, ch, yus)
        emit_norm_a(*pending_norm)
        emit_norm(*pending_norm)
        pending_norm = None
        psA.release()
        p_yu.release()
        p_sty.release()
        p_ex.release()
        p_att.release()
        p_vsb.release()

        if phases == 2:
            nc.sync.dma_start(out=out_d[:, :, 0 : NO // 4], in_=yTn8[:].bitcast(f32))
            for p in (p_w, p_qk, p_w8, p_w2h, p_y, stats, consts, dram):
                p.release()
            return

        # ====== Phase P6: Wo + residual, LN2 stats under the matmuls ======
        p_res = tc.alloc_tile_pool(name="p_res", bufs=1, side="right")
        x2T = p_res.tile([P, DT, NO], f32, tag="x2T")
        xn2T = p_res.tile([P, DT, NO], f8, tag="xn2T")

        ps6 = tc.alloc_tile_pool(name="ps6", bufs=3, space="PSUM")
        nc.sync.dma_start(out=w2_sb[:, 0 : HT // 2, :], in_=w2_d[:, 0 : HT // 2, :])
        nc.sync.dma_start(out=w2_sb[:, HT // 2 :, :], in_=w2_d[:, HT // 2 :, :])
        # all 24 w1 slices land during Wo + the collective, and both MLP
        # chunks reuse them (no refetch)
        w1_tiles = []
        for kh in range(HT):
            t = p_w8.tile([P, DT, P], f8, tag="w1s", bufs=HT, name="w1s")
            nc.scalar.dma_start(out=t[:], in_=w1_d[:, :, kh * P : (kh + 1) * P])
            w1_tiles.append(t)

        mvs2 = stats.tile([P, DT, 2], f32, tag="mvs2")
        bnst2 = stats.tile([P, OCH, nc.vector.BN_STATS_DIM], f32, tag="bnst2")
        st = stats.tile([P, DT, 2], f32, tag="st")

        # lag-2 software pipeline: each group's first two dk-pairs are
        # emitted ahead, so the dk=(4,5) matmuls (gated by the last head
        # pair's normalize) stall less
        wo_ps = {}

        def wo_head(g):
            dt, ch = g
            ps = ps6.tile([P, CH], f32, tag="ps", name="ps6t")
            wo_ps[g] = ps
            for dk in (0, 2):
                nc.tensor.matmul(
                    ps[:],
                    lhsT=wo_sb[:, dk : dk + 2, dt * P : (dt + 1) * P],
                    rhs=yTn8[:, dk : dk + 2, ch * CH : (ch + 1) * CH],
                    start=(dk == 0),
                    stop=False,
                    perf_mode=DR,
                )

        def wo_tail(g):
            dt, ch = g
            ps = wo_ps.pop(g)
            nc.tensor.matmul(
                ps[:],
                lhsT=wo_sb[:, 4:6, dt * P : (dt + 1) * P],
                rhs=yTn8[:, 4:6, ch * CH : (ch + 1) * CH],
                start=False,
                stop=False,
                perf_mode=DR,
            )
            nc.tensor.matmul(
                ps[:],
                lhsT=id_sb[:],
                rhs=xo131[:, dt, ch * CH : (ch + 1) * CH],
                start=False,
                stop=True,
            )
            sl = (slice(None), dt, slice(ch * CH, (ch + 1) * CH))
            nc.vector.tensor_scalar_mul(
                out=x2T[sl], in0=ps[:], scalar1=1.0 / (SW * SY)
            )
            nc.vector.bn_stats(out=bnst2[:, ch, :], in_=x2T[sl])
            if ch == OCH - 1:
                nc.vector.bn_aggr(out=mvs2[:, dt, :], in_=bnst2[:])

        wo_pend = deque()
        for g in [(dt, ch) for dt in range(DT) for ch in range(OCH)]:
            wo_head(g)
            wo_pend.append(g)
            if len(wo_pend) > 2:
                wo_tail(wo_pend.popleft())
        while wo_pend:
            wo_tail(wo_pend.popleft())
        # partial sums for the pairwise AllReduce:
        # sum = NO*mean ; sumsq = NO*var_pop + sum*mean
        nc.vector.tensor_scalar_mul(out=st[:, :, 0], in0=mvs2[:, :, 0], scalar1=float(NO))
        nc.vector.tensor_mul(out=st[:, :, 1], in0=st[:, :, 0], in1=mvs2[:, :, 0])
        # st1 = var*NO + st1(=sum*mean)
        tmp = stats.tile([P, DT], f32, tag="tmpv")
        nc.vector.tensor_scalar_mul(out=tmp[:], in0=mvs2[:, :, 1], scalar1=float(NO))
        nc.vector.tensor_add(out=st[:, :, 1], in0=st[:, :, 1], in1=tmp[:])

        ps6.release()


        # ====== Phase P7: LN2 (pairwise AllReduce of partial sums) ======
        nc.gpsimd.dma_start(out=cc_in[:], in_=st[:])
        if sim:
            nc.gpsimd.dma_start(out=cc_out[:], in_=cc_in[:])
        else:
            nc.gpsimd.collective_compute(
                "AllReduce",
                ALU.add,
                replica_groups=[[0, 1], [2, 3], [4, 5], [6, 7]],
                ins=[cc_in.opt()],
                outs=[cc_out.opt()],
            )
        stf = stats.tile([P, DT, 2], f32, tag="stf")
        nc.gpsimd.dma_start(out=stf[:], in_=cc_out[:])

        mu = stats.tile([P, DT], f32, tag="mu")
        sg2 = stats.tile([P, DT], f32, tag="sg2")
        in2 = stats.tile([P, DT], f32, tag="in2")
        sc2 = stats.tile([P, DT], f32, tag="sc2")
        bi2 = stats.tile([P, DT], f32, tag="bi2")
        nc.vector.tensor_scalar_mul(out=mu[:], in0=stf[:, :, 0], scalar1=1.0 / N)
        nc.vector.tensor_mul(out=sg2[:], in0=mu[:], in1=stf[:, :, 0])
        nc.vector.tensor_tensor(
            out=sg2[:], in0=stf[:, :, 1], in1=sg2[:], op=ALU.subtract
        )
        nc.scalar.activation(
            out=sg2[:], in_=sg2[:], func=AF.Sqrt, scale=1.0 / (N - 1)
        )
        nc.vector.reciprocal(out=in2[:], in_=sg2[:])
        nc.vector.tensor_mul(out=sc2[:], in0=ln2w, in1=in2[:])
        nc.vector.tensor_mul(out=bi2[:], in0=mu[:], in1=sc2[:])
        nc.vector.tensor_tensor(out=bi2[:], in0=ln2b, in1=bi2[:], op=ALU.subtract)

        if phases == 3:
            for dt in range(DT):
                nc.vector.tensor_scalar(
                    out=xn2T[:, dt, :],
                    in0=x2T[:, dt, :],
                    scalar1=sc2[:, dt : dt + 1],
                    scalar2=bi2[:, dt : dt + 1],
                    op0=ALU.mult,
                    op1=ALU.add,
                )
            nc.sync.dma_start(
                out=out_d[:, :, 0 : NO // 4], in_=xn2T[:].bitcast(f32)
            )
            for p in (p_w8, p_w2h, p_res, p_w, p_qk, p_y, stats, consts, dram):
                p.release()
            return

        # ========== Phase P8: MLP (hold w2, stream w1 slices) ==========
        ps8 = tc.alloc_tile_pool(name="ps8", bufs=1, space="PSUM")
        for ch in range(OCH):
            csl = slice(ch * CH, (ch + 1) * CH)
            for dt in range(DT):
                eng = nc.vector if ch == 0 else nc.gpsimd
                eng.tensor_scalar(
                    out=xn2T[:, dt, csl],
                    in0=x2T[:, dt, csl],
                    scalar1=sc2[:, dt : dt + 1],
                    scalar2=bi2[:, dt : dt + 1],
                    op0=ALU.mult,
                    op1=ALU.add,
                )
        for ch in range(OCH):
            csl = slice(ch * CH, (ch + 1) * CH)
            xop = [
                ps8.tile([P, CH], f32, tag=f"xop{dt}", bufs=1, name=f"xop{dt}")
                for dt in range(DT)
            ]
            for khp in range(HTP):
                hkp = p_w8.tile([P, 2, CH], f8, tag="hk", name="hk")
                for j in range(2):
                    kh = 2 * khp + j
                    w1s = w1_tiles[kh]
                    hp = ps8.tile([P, CH], f32, tag="hp", bufs=2, name="hp")
                    for dk in range(0, DT, 2):
                        nc.tensor.matmul(
                            hp[:],
                            lhsT=w1s[:, dk : dk + 2, :],
                            rhs=xn2T[:, dk : dk + 2, csl],
                            start=(dk == 0),
                            stop=(dk == DT - 2),
                            perf_mode=DR,
                        )
                    nc.scalar.activation(
                        out=hkp[:, j, :],
                        in_=hp[:],
                        func=AF.Gelu,
                        bias=b1_[:, kh : kh + 1],
                        scale=1.0 / SW,
                    )
                for dt in range(DT):
                    nc.tensor.matmul(
                        xop[dt][:],
                        lhsT=w2_sb[:, 2 * khp : 2 * khp + 2, dt * P : (dt + 1) * P],
                        rhs=hkp[:],
                        start=(khp == 0),
                        stop=(khp == HTP - 1),
                        perf_mode=DR,
                    )
            # bias b2'' + residual, stream out per-slice
            for dt in range(DT):
                sl = (slice(None), dt, csl)
                osb = p_w8.tile([P, CH], f32, tag="osb", name="osb")
                nc.scalar.activation(
                    out=osb[:],
                    in_=xop[dt][:],
                    func=AF.Identity,
                    bias=b2pp[:, dt : dt + 1],
                    scale=1.0 / SW,
                )
                aeng = nc.gpsimd if dt % 2 == 0 else nc.vector
                aeng.tensor_add(out=x2T[sl], in0=x2T[sl], in1=osb[:])
                deng = (nc.sync, nc.scalar, nc.gpsimd)[dt % 3]
                deng.dma_start(out=out_d[:, dt, csl], in_=x2T[sl])

        ps8.release()
        for p in (p_w8, p_w2h, p_res, p_w, p_qk, p_y, stats, consts, dram):
            p.release()

    for _rep in range(reps):
        with tile.TileContext(nc) as tc:
            with nc.allow_low_precision(
                reason="fp8 weights/activations; fp32 residual"
            ):
                body(tc)
    _split_waits(nc, mybir)
    return nc


def _feat_tiles(a, dt=np.float32):
    """[D_in, ...] -> [P, D_in//P, ...] with feature f = dt*P + p."""
    return np.ascontiguousarray(
        a.reshape(a.shape[0] // P, P, *a.shape[1:])
        .transpose(1, 0, *range(2, a.ndim + 1))
        .astype(dt)
    )


def _q8(a):
    import ml_dtypes

    return np.clip(np.asarray(a, np.float32), -240, 240).astype(
        ml_dtypes.float8_e4m3
    )


def _prep_inputs(x, ln1_w, ln1_b, ln2_w, ln2_b, wq, bq, wk, bk, wv, bv, wo, bo, w1, b1, w2, b2):
    import ml_dtypes

    f = np.float32
    bf = ml_dtypes.bfloat16
    sel = np.zeros((12, 12, 64), f)
    for h in range(12):
        sel[h, h, :] = SY
    b2pp = (
        np.asarray(b2, f)
        + np.asarray(bo, f)
        + np.asarray(bv, f) @ np.asarray(wo, f)
    )
    vecs = np.zeros((P, 8, DT), f)
    for i, v in enumerate(
        (ln1_w, ln1_b, ln2_w, ln2_b, np.asarray(bq, f) / np.sqrt(f(D)), bk,
         np.zeros(D, f), b2pp)
    ):
        vecs[:, i, :] = np.asarray(v, f).reshape(DT, P).T
    shared = {
        "wq8": _q8(_feat_tiles(np.asarray(wq, f) * SW)),
        "wk8": _q8(_feat_tiles(np.asarray(wk, f) * SW)),
        "wv8": _q8(_feat_tiles(np.asarray(wv, f) * SW)),
        "wo8": _q8(_feat_tiles(np.asarray(wo, f) * SW)),
        "w18": _q8(_feat_tiles(np.asarray(w1, f) * SW)),
        "w28": _q8(_feat_tiles(np.asarray(w2, f) * SW)),
        "vecs": vecs,
        "b1": np.ascontiguousarray(np.asarray(b1, f).reshape(HT, P).T),
        "sel": sel,
        "idm": np.eye(P, dtype=bf),
    }
    in_maps = []
    for c in range(NC):
        b, half = c // 2, c % 2
        xb = np.asarray(x[b], f)
        own = xb[half * NO : (half + 1) * NO]
        oth = xb[(1 - half) * NO : (2 - half) * NO]
        xTc = np.concatenate([own, oth], axis=0).T  # [D, N], own rows first
        m = dict(shared)
        m["xT"] = _feat_tiles(np.ascontiguousarray(xTc), bf)
        m["xo131"] = _feat_tiles(
            np.ascontiguousarray(own.T) * (SW * SY), bf
        )
        in_maps.append(m)
    return in_maps


def _assemble(results):
    out = np.empty((B, N, D), np.float32)
    for c in range(NC):
        b, half = c // 2, c % 2
        oT = results[c]["outT"]  # [P, DT, NO]
        out[b, half * NO : (half + 1) * NO] = (
            oT.transpose(1, 0, 2).reshape(D, NO).T
        )
    return out


def run_kernel_raw(inputs, **spmd_kwargs):
    from concourse.bass_utils import run_bass_kernel_spmd

    if "nc" not in _CACHE:
        _CACHE["nc"] = _build_bass()
    nc = _CACHE["nc"]
    in_maps = _prep_inputs(**inputs)
    res = run_bass_kernel_spmd(nc, in_maps, core_ids=list(range(NC)), **spmd_kwargs)
    return _assemble(res.results), res


def kernel(**inputs):
    out, _ = run_kernel_raw(inputs)
    return out
